# revision 28
# baseline (speedup 1.0000x reference)
"""Trainium2 Bass kernel for CustomMultiheadAttention.

Problem shapes: query/key/value [2048, 4, 1024] f32, causal mask [2048, 2048],
Wq/Wk/Wv/Wo [1024, 1024] (torch Linear layout [out, in]), biases [1024].
16 heads, head dim 64.

Sharding over 8 cores: core c -> (batch b = c // 2, head-group hg = c % 2).
Each core computes 8 heads (an E-slice of 512 rows of Wq/Wk/Wv, 512 cols of
Wo) for one batch. Host sums the two partial output projections per batch and
adds bo.

Device schedule (v2 — PE-tiling-aware rewrite of the baseline):
  - Projections run 128x128-mode with DMAs chunked/ordered so the first
    matmul starts ~6us in; a short junk-matmul burst warms the HAM clock
    gate during the initial DMA wait.
  - Attention processes head PAIRS: head A lives on SBUF partitions 0-63,
    head B on 64-127. Score matmuls (K=64) for A/B are issued alternately
    with tile_position (0,0)/(64,0) so the two 64x128 row-tiles of the PE
    array stream CONCURRENTLY (~2x score throughput).
  - exp on ACT is the attention-phase bottleneck; PV (K=128 mode), the
    output projection, and normalization are kept in a deferred FIFO of
    128-mode work units popped one per score-group into the ACT-bound gaps,
    so both PE and ACT stay saturated and PE tiling modes switch at group
    (not instruction) granularity.
  - Normalize: den row 64 of PV out; reciprocal_approx_fast on DVE (~5x
    faster than exact reciprocal; den >= 1 so edge cases are impossible),
    gpsimd partition_broadcast, DVE multiply -> attnT bf16.
"""

import math
import os
import sys

import numpy as np

for _p in ("/opt/trn_rl_repo", os.path.expanduser("~/.axon_site/_ro/trn_rl_repo")):
    if os.path.isdir(_p) and _p not in sys.path:
        sys.path.insert(0, _p)

import ml_dtypes  # noqa: E402

import concourse.bass as bass  # noqa: E402
import concourse.tile as tile  # noqa: E402
from concourse import bacc, bass_utils, library_config, mybir  # noqa: E402

# Problem constants
T, S, B, E, H = 2048, 2048, 4, 1024, 16
D = E // H  # 64
NCORES = 8
HC = H // 2  # heads per core
EH = HC * D  # 512 per-core E-slice
P = 128
TC = 512  # t-chunk
NT = T // TC  # 4
NSB = S // P  # 16 s-blocks
KO = E // P  # 8 contraction chunks for projections
KHD = EH // P  # 4 contraction chunks for out proj
VW = D + 1  # 65: head V width incl ones column
NPAIR = KHD  # 4 head pairs per core
BF16 = mybir.dt.bfloat16
F32 = mybir.dt.float32
NPBF16 = ml_dtypes.bfloat16

_CACHE: dict = {}


def _build_nc():
    nc = bacc.Bacc(
        "TRN2",
        target_bir_lowering=False,
        debug=False,
        enable_asserts=True,
        num_devices=NCORES,
    )
    AF = mybir.ActivationFunctionType

    xq_t = nc.dram_tensor("xq_t", [E, T], BF16, kind="ExternalInput").ap()
    xk_t = nc.dram_tensor("xk_t", [E, T], BF16, kind="ExternalInput").ap()
    xv_t = nc.dram_tensor("xv_t", [E, T], BF16, kind="ExternalInput").ap()
    wq_t = nc.dram_tensor("wq_t", [E, EH], BF16, kind="ExternalInput").ap()
    wk_t = nc.dram_tensor("wk_t", [E, EH], BF16, kind="ExternalInput").ap()
    wv_t = nc.dram_tensor("wv_t", [E, EH], BF16, kind="ExternalInput").ap()
    wo_t = nc.dram_tensor("wo_t", [EH, E], BF16, kind="ExternalInput").ap()
    bq_d = nc.dram_tensor("bq_d", [P, KHD], F32, kind="ExternalInput").ap()
    bk_d = nc.dram_tensor("bk_d", [P, KHD], F32, kind="ExternalInput").ap()
    bv_d = nc.dram_tensor("bv_d", [P, EH], F32, kind="ExternalInput").ap()
    mask_d = nc.dram_tensor("mask_d", [P, 4, TC], BF16, kind="ExternalInput").ap()
    out_p = nc.dram_tensor("out_part", [T, E], F32, kind="ExternalOutput").ap()

    from contextlib import ExitStack

    with tile.TileContext(nc) as tc, ExitStack() as ctx:
        persist = ctx.enter_context(tc.tile_pool(name="persist", bufs=1))
        evac = ctx.enter_context(tc.tile_pool(name="evac", bufs=4))

        nc.gpsimd.load_library(library_config.attn)  # for partition_broadcast

        # ---- persistent tiles
        wq_sb = persist.tile([P, KO, EH], BF16, tag="wq")
        wk_sb = persist.tile([P, KO, EH], BF16, tag="wk")
        wv_sb = persist.tile([P, KO, EH], BF16, tag="wv")
        bq_sb = persist.tile([P, KHD], F32, tag="bq")
        bk_sb = persist.tile([P, KHD], F32, tag="bk")
        bv_sb = persist.tile([P, EH], F32, tag="bv")
        mask_sb = persist.tile([P, 4, TC], BF16, tag="mask")
        wo_sb = persist.tile([P, KHD, E], BF16, tag="wo")
        qt_sb = persist.tile([P, KHD, T], BF16, tag="qt")
        kt_sb = persist.tile([P, KHD, T], BF16, tag="kt")
        v_sb = persist.tile([P, NSB, HC * VW], BF16, tag="v")
        attnT = persist.tile([P, KHD, T], BF16, tag="attnT")
        warm_sb = persist.tile([P, TC], BF16, tag="warm")

        # DMA queue round-robin across sync + scalar trigger queues
        dma_rr = [0]

        def dma(dst, src):
            eng = nc.sync if dma_rr[0] % 2 == 0 else nc.scalar
            dma_rr[0] += 1
            eng.dma_start(dst, src)

        wq_src = wq_t.rearrange("(ko p) m -> p ko m", p=P)
        wk_src = wk_t.rearrange("(ko p) m -> p ko m", p=P)
        wv_src = wv_t.rearrange("(ko p) m -> p ko m", p=P)
        xq_src = xq_t.rearrange("(ko p) t -> p ko t", p=P)
        xk_src = xk_t.rearrange("(ko p) t -> p ko t", p=P)
        xv_src = xv_t.rearrange("(ko p) t -> p ko t", p=P)
        wo_src = wo_t.rearrange("(ko p) m -> p ko m", p=P)

        # ---- single fused phase: attention score groups stream in
        # 64-row-tile mode (head pairs concurrent on PE row-tiles T0/T8);
        # ALL 128-mode matmul work (Q/K/V projections, PV accumulation,
        # out-projection) flows through a deferred FIFO popped between score
        # groups, keeping both PE and ACT saturated from ~10us onward.
        with (
            tc.tile_pool(name="xu", bufs=3) as xu,
            tc.tile_pool(name="exps", bufs=16) as exps,
            tc.tile_pool(name="psS", bufs=2, space="PSUM") as psS,
            tc.tile_pool(name="psPV", bufs=2, space="PSUM") as psPV,
        ):
            pe128 = []  # deferred FIFO of (unit_id, emit_fn)
            emitted = set()
            uid_counter = [0]

            def push(fn):
                uid = uid_counter[0]
                uid_counter[0] += 1
                pe128.append((uid, fn))
                return uid

            def pop_unit():
                uid, fn = pe128.pop(0)
                fn()
                emitted.add(uid)

            def pop_to(depth):
                # Keep the deferred queue bounded so PSUM/SBUF ring reuse
                # never outruns the emission of a tile's consumers.
                while len(pe128) > depth:
                    pop_unit()

            def drain_to(uid):
                while uid not in emitted:
                    pop_unit()

            def make_proj_unit(pair, tj, x_src, w_sb, b_sb, dst):
                def _emit():
                    xc = xu.tile([P, KO, TC], BF16, tag="xc")
                    for ko in range(KO):  # per-ko DMAs parallelize across engines
                        dma(xc[:, ko, :], x_src[:, ko, tj * TC : (tj + 1) * TC])
                    ps = psPV.tile([P, TC], F32, tag="po")
                    for ko in range(KO):
                        nc.tensor.matmul(
                            ps[:],
                            lhsT=w_sb[:, ko, pair * P : (pair + 1) * P],
                            rhs=xc[:, ko, :],
                            start=(ko == 0),
                            stop=(ko == KO - 1),
                        )
                    nc.vector.tensor_scalar_add(
                        dst[:, pair, tj * TC : (tj + 1) * TC],
                        ps[:],
                        b_sb[:, pair : pair + 1],
                    )

                return _emit

            def make_v_unit(sb):
                def _emit():
                    xc = xu.tile([P, KO, P], BF16, tag="xv")
                    for ko in range(KO):
                        dma(xc[:, ko, :], xv_src[:, ko, sb * P : (sb + 1) * P])
                    ps = psPV.tile([P, EH], F32, tag="po")
                    for ko in range(KO):
                        nc.tensor.matmul(
                            ps[:],
                            lhsT=xc[:, ko, :],
                            rhs=wv_sb[:, ko, :],
                            start=(ko == 0),
                            stop=(ko == KO - 1),
                        )
                    v_dst = v_sb[:, sb, :].rearrange("p (h x) -> p h x", h=HC)[
                        :, :, 0:D
                    ]
                    nc.vector.tensor_add(
                        v_dst,
                        ps[:].rearrange("p (h x) -> p h x", h=HC),
                        bv_sb[:].rearrange("p (h x) -> p h x", h=HC),
                    )

                return _emit

            def make_pv_pair(pvA, pvB, si, hA, hB, ets, tj):
                last_si = 4 * tj + 3

                def _emit():
                    et = ets.pop(si)
                    for pv, h, u in ((pvA, hA, 0), (pvB, hB, 1)):
                        nc.tensor.matmul(
                            pv[0 : D + 1, :],
                            lhsT=v_sb[:, si, h * VW : (h + 1) * VW],
                            rhs=et[:, u, :],
                            start=(si == 0),
                            stop=(si == last_si),
                        )

                return _emit

            def make_norm(pv, pb, pair, tj):
                def _emit():
                    # reciprocal_approx_fast reads partition 0 regardless of
                    # the input AP's base partition, so stage the den row
                    # (PSUM partition 64) to a partition-0 tile first.
                    den0 = evac.tile([1, TC], F32, tag="den0", bufs=2)
                    nc.vector.tensor_copy(den0[:], pv[D : D + 1, :])
                    rec = evac.tile([1, TC], F32, tag="rec", bufs=2)
                    nc.vector.reciprocal_approx_fast(rec[:], den0[:])
                    rbs = evac.tile([D, TC], F32, tag="rbs", bufs=2)
                    nc.gpsimd.partition_broadcast(rbs[:], rec[:])
                    nc.vector.tensor_mul(
                        attnT[pb : pb + D, pair, tj * TC : (tj + 1) * TC],
                        pv[0:D, :],
                        rbs[:],
                    )

                return _emit

            def make_outproj(tb, fj):
                def _emit():
                    po = psPV.tile([P, TC], F32, tag="po")
                    for ko in range(KHD):
                        nc.tensor.matmul(
                            po[:],
                            lhsT=attnT[:, ko, tb * P : (tb + 1) * P],
                            rhs=wo_sb[:, ko, fj * TC : (fj + 1) * TC],
                            start=(ko == 0),
                            stop=(ko == KHD - 1),
                        )
                    ot = evac.tile([P, TC], F32, tag="ot", bufs=3)
                    nc.vector.tensor_copy(ot[:], po[:])
                    nc.gpsimd.dma_start(
                        out_p[tb * P : (tb + 1) * P, fj * TC : (fj + 1) * TC], ot[:]
                    )

                return _emit

            # startup: warm the HAM clock gate with junk matmuls while the
            # first weight DMAs land; interleave the first Q/K units with
            # the weight-DMA emission so their x transfers queue early.
            nc.vector.memset(warm_sb[:], 0.0)
            wps = psPV.tile([P, TC], F32, tag="po")
            for i in range(8):
                nc.tensor.matmul(
                    wps[:],
                    lhsT=warm_sb[:, 0:P],
                    rhs=warm_sb[:],
                    start=True,
                    stop=True,
                )
            for h in range(HC):
                nc.vector.memset(v_sb[:, :, h * VW + D : h * VW + VW], 1.0)

            batches = [(tj, pair) for tj in range(NT) for pair in range(NPAIR)]
            qk_marker = {}
            for ko in range(KO):
                dma(wq_sb[:, ko, :], wq_src[:, ko, :])
            dma(bq_sb[:], bq_d)
            uid_q0 = push(make_proj_unit(0, 0, xq_src, wq_sb, bq_sb, qt_sb))
            drain_to(uid_q0)
            for ko in range(KO):
                dma(wk_sb[:, ko, :], wk_src[:, ko, :])
            dma(bk_sb[:], bk_d)
            uid_k0 = push(make_proj_unit(0, 0, xk_src, wk_sb, bk_sb, kt_sb))
            drain_to(uid_k0)
            qk_marker[0] = (uid_q0, uid_k0)
            for ko in range(KO):
                dma(wv_sb[:, ko, :], wv_src[:, ko, :])
            dma(bv_sb[:], bv_d)
            dma(mask_sb[:], mask_d)
            for sb in range(4):
                push(make_v_unit(sb))
            for ko in range(KHD):
                dma(wo_sb[:, ko, :], wo_src[:, ko, :])

            for b, (tj, pair) in enumerate(batches):
                ng2 = 4 * tj + 4  # s-blocks in the causal span of this tj
                hA, hB = 2 * pair, 2 * pair + 1
                if True:
                    # push the NEXT batch's Q/K prereqs (one batch of lead so
                    # their x DMAs land before the matmuls need them), plus
                    # one V unit of the NEXT tj band
                    if b + 1 < len(batches):
                        tjn, pn = batches[b + 1]
                        qk_marker[b + 1] = (
                            push(make_proj_unit(pn, tjn, xq_src, wq_sb, bq_sb, qt_sb)),
                            push(make_proj_unit(pn, tjn, xk_src, wk_sb, bk_sb, kt_sb)),
                        )
                    sbv = 4 * tj + 4 + pair
                    if sbv < NSB:
                        push(make_v_unit(sbv))
                    # ensure this batch's own Q/K projections are emitted
                    for uid in qk_marker[b]:
                        drain_to(uid)
                    pvA = psPV.tile([P, TC], F32, tag="pv")
                    pvB = psPV.tile([P, TC], F32, tag="pv")
                    ets: dict = {}
                    for si in range(ng2):
                        # one group = one s-block for BOTH heads of the pair:
                        # a single PSUM tile + a single exp gate both next
                        # matmuls, so the scheduler keeps the A/B row-tile
                        # matmuls adjacent and they stream concurrently.
                        sc = psS.tile([P, 2, TC], F32, tag="sc")
                        for pb, u in ((0, 0), (D, 1)):
                            nc.tensor.matmul(
                                sc[:, u, :],
                                lhsT=kt_sb[pb : pb + D, pair, si * P : (si + 1) * P],
                                rhs=qt_sb[pb : pb + D, pair, tj * TC : (tj + 1) * TC],
                                start=True,
                                stop=True,
                            )
                        et = exps.tile([P, 2, TC], BF16, tag="et")
                        k = si - 4 * tj
                        if k >= 0:  # diagonal s-block: partial-span exp + mask
                            if k > 0:
                                nc.gpsimd.memset(et[:, :, 0 : P * k], 0.0)
                            nc.scalar.activation(
                                et[:, :, P * k : TC],
                                sc[:, :, P * k : TC],
                                AF.Exp,
                                scale=1.0 / math.sqrt(D),
                            )
                            for u in range(2):
                                nc.vector.tensor_mul(
                                    et[:, u, P * k : P * (k + 1)],
                                    et[:, u, P * k : P * (k + 1)],
                                    mask_sb[:, k, P * k : P * (k + 1)],
                                )
                        else:
                            nc.scalar.activation(
                                et[:], sc[:], AF.Exp, scale=1.0 / math.sqrt(D)
                            )
                        ets[si] = et
                        push(make_pv_pair(pvA, pvB, si, hA, hB, ets, tj))
                        # pop deferred 128-mode work in small batches (batching
                        # limits PE tiling-mode switches between the 64-row
                        # score stream and the 128-row PV/out-proj stream)
                        if si % 3 == 2:
                            pop_to(3)
                    push(make_norm(pvA, 0, pair, tj))
                    push(make_norm(pvB, D, pair, tj))
                if pair == NPAIR - 1:
                    # queue this tj's out-projection tiles; they run inside tj+1
                    for tb in range(4 * tj, 4 * tj + 4):
                        for fj in range(E // TC):
                            push(make_outproj(tb, fj))
            pop_to(0)

    nc.compile()
    return nc


def _get_nc():
    if "nc" not in _CACHE:
        _CACHE["nc"] = _build_nc()
    return _CACHE["nc"]


def _prep_in_maps(query, key, value, attn_mask, Wq, bq, Wk, bk, Wv, bv, Wo, bo):
    """Host-side prep: slices, transposes, bf16 casts. Returns in_maps[8]."""
    f32 = np.float32
    xt = {}  # (kind, b) -> [E, T] bf16
    for b in range(B):
        xt[("q", b)] = np.ascontiguousarray(query[:, b, :].T).astype(NPBF16)
        xt[("k", b)] = np.ascontiguousarray(key[:, b, :].T).astype(NPBF16)
        xt[("v", b)] = np.ascontiguousarray(value[:, b, :].T).astype(NPBF16)
    wt = {}
    for hg in range(2):
        sl = slice(EH * hg, EH * hg + EH)
        wt[("q", hg)] = np.ascontiguousarray(Wq[sl, :].T).astype(NPBF16)
        wt[("k", hg)] = np.ascontiguousarray(Wk[sl, :].T).astype(NPBF16)
        wt[("v", hg)] = np.ascontiguousarray(Wv[sl, :].T).astype(NPBF16)
        wt[("o", hg)] = np.ascontiguousarray(Wo[:, sl].T).astype(NPBF16)
        wt[("bq", hg)] = np.ascontiguousarray(
            bq[sl].astype(f32).reshape(KHD, P).T
        )
        wt[("bk", hg)] = np.ascontiguousarray(
            bk[sl].astype(f32).reshape(KHD, P).T
        )
        wt[("bv", hg)] = np.ascontiguousarray(
            np.tile(bv[sl].astype(f32)[None, :], (P, 1))
        )
    # mask patterns: for a scores tile with s0 = t0 + 128*o, pattern
    # [p, o, f] = 0 if attn_mask[t0+f, s0+p] (masked) else 1.
    t0 = 512
    patts = []
    for o in range(4):
        s0 = t0 + P * o
        patts.append(
            (~np.asarray(attn_mask[t0 : t0 + TC, s0 : s0 + P])).T.astype(NPBF16)
        )
    mask_tiles = np.ascontiguousarray(np.stack(patts, axis=1))  # [P, 4, TC]

    in_maps = []
    for c in range(NCORES):
        b, hg = c // 2, c % 2
        in_maps.append(
            {
                "xq_t": xt[("q", b)],
                "xk_t": xt[("k", b)],
                "xv_t": xt[("v", b)],
                "wq_t": wt[("q", hg)],
                "wk_t": wt[("k", hg)],
                "wv_t": wt[("v", hg)],
                "wo_t": wt[("o", hg)],
                "bq_d": wt[("bq", hg)],
                "bk_d": wt[("bk", hg)],
                "bv_d": wt[("bv", hg)],
                "mask_d": mask_tiles,
            }
        )
    return in_maps


def _run_on_hw(in_maps, trace=False, **kwargs):
    nc = _get_nc()
    return bass_utils.run_bass_kernel_spmd(
        nc, in_maps, core_ids=list(range(NCORES)), trace=trace, **kwargs
    )


def _gather(results, bo):
    outs = []
    for b in range(B):
        part = results[2 * b]["out_part"] + results[2 * b + 1]["out_part"]
        outs.append(part)
    out = np.stack(outs, axis=1)  # [T, B, E]
    out += np.asarray(bo, dtype=np.float32)[None, None, :]
    return out.astype(np.float32)


def _numpy_fallback(query, key, value, attn_mask, Wq, bq, Wk, bk, Wv, bv, Wo, bo):
    """Exact f32 numpy replication of the reference (for non-causal masks)."""
    f32 = np.float32
    query, key, value = (np.asarray(a, f32) for a in (query, key, value))
    q = (np.einsum("tbe,fe->btf", query, Wq, dtype=f32) + bq).reshape(B, T, H, D)
    k = (np.einsum("sbe,fe->bsf", key, Wk, dtype=f32) + bk).reshape(B, S, H, D)
    v = (np.einsum("sbe,fe->bsf", value, Wv, dtype=f32) + bv).reshape(B, S, H, D)
    q, k, v = (a.transpose(0, 2, 1, 3) for a in (q, k, v))
    out = np.empty((B, H, T, D), f32)
    mask = np.asarray(attn_mask)
    for b in range(B):
        for h in range(H):
            sc = (q[b, h] @ k[b, h].T) / np.float32(math.sqrt(D))
            sc = np.where(mask, -np.inf, sc)
            m = np.max(sc, axis=-1, keepdims=True)
            m = np.where(np.isfinite(m), m, 0.0)
            e = np.exp(sc - m)
            p = e / np.sum(e, axis=-1, keepdims=True)
            p = np.where(np.isinf(sc), 0.0, p)
            out[b, h] = p @ v[b, h]
    out = out.transpose(0, 2, 1, 3).reshape(B, T, E)
    out = out @ np.asarray(Wo, f32).T + bo
    return np.ascontiguousarray(out.transpose(1, 0, 2)).astype(f32)


def kernel(query, key, value, attn_mask, Wq, bq, Wk, bk, Wv, bv, Wo, bo):
    mask = np.asarray(attn_mask)
    causal = mask.shape == (T, S) and np.array_equal(
        mask, np.triu(np.ones((T, S), dtype=bool), k=1)
    )
    if not causal:
        return _numpy_fallback(
            query, key, value, attn_mask, Wq, bq, Wk, bk, Wv, bv, Wo, bo
        )
    in_maps = _prep_in_maps(
        query, key, value, attn_mask, Wq, bq, Wk, bk, Wv, bv, Wo, bo
    )
    res = _run_on_hw(in_maps)
    return _gather(res.results, bo)


# revision 33
# speedup vs baseline: 1.1704x; 1.1704x over previous
"""Trainium2 Bass kernel for CustomMultiheadAttention.

Problem shapes: query/key/value [2048, 4, 1024] f32, causal mask [2048, 2048],
Wq/Wk/Wv/Wo [1024, 1024] (torch Linear layout [out, in]), biases [1024].
16 heads, head dim 64.

Sharding over 8 cores: core c -> (batch b = c // 2, head-group hg = c % 2).
Each core computes 8 heads (an E-slice of 512 rows of Wq/Wk/Wv, 512 cols of
Wo) for one batch. Host sums the two partial output projections per batch and
adds bo.

Device schedule (v2 — PE-tiling-aware rewrite of the baseline):
  - Projections run 128x128-mode with DMAs chunked/ordered so the first
    matmul starts ~6us in; a short junk-matmul burst warms the HAM clock
    gate during the initial DMA wait.
  - Attention processes head PAIRS: head A lives on SBUF partitions 0-63,
    head B on 64-127. Score matmuls (K=64) for A/B are issued alternately
    with tile_position (0,0)/(64,0) so the two 64x128 row-tiles of the PE
    array stream CONCURRENTLY (~2x score throughput).
  - exp on ACT is the attention-phase bottleneck; PV (K=128 mode), the
    output projection, and normalization are kept in a deferred FIFO of
    128-mode work units popped one per score-group into the ACT-bound gaps,
    so both PE and ACT stay saturated and PE tiling modes switch at group
    (not instruction) granularity.
  - Normalize: den row 64 of PV out; reciprocal_approx_fast on DVE (~5x
    faster than exact reciprocal; den >= 1 so edge cases are impossible),
    gpsimd partition_broadcast, DVE multiply -> attnT bf16.
"""

import math
import os
import sys

import numpy as np

for _p in ("/opt/trn_rl_repo", os.path.expanduser("~/.axon_site/_ro/trn_rl_repo")):
    if os.path.isdir(_p) and _p not in sys.path:
        sys.path.insert(0, _p)

import ml_dtypes  # noqa: E402

import concourse.bass as bass  # noqa: E402
import concourse.tile as tile  # noqa: E402
from concourse import bacc, bass_utils, library_config, mybir  # noqa: E402

# Problem constants
T, S, B, E, H = 2048, 2048, 4, 1024, 16
D = E // H  # 64
NCORES = 8
HC = H // 2  # heads per core
EH = HC * D  # 512 per-core E-slice
P = 128
TC = 512  # t-chunk
NT = T // TC  # 4
NSB = S // P  # 16 s-blocks
KO = E // P  # 8 contraction chunks for projections
KHD = EH // P  # 4 contraction chunks for out proj
VW = D + 1  # 65: head V width incl ones column
NPAIR = KHD  # 4 head pairs per core
BF16 = mybir.dt.bfloat16
F32 = mybir.dt.float32
NPBF16 = ml_dtypes.bfloat16

_CACHE: dict = {}


def _build_nc():
    nc = bacc.Bacc(
        "TRN2",
        target_bir_lowering=False,
        debug=False,
        enable_asserts=True,
        num_devices=NCORES,
    )
    AF = mybir.ActivationFunctionType

    xq_t = nc.dram_tensor("xq_t", [E, T], BF16, kind="ExternalInput").ap()
    xk_t = nc.dram_tensor("xk_t", [E, T], BF16, kind="ExternalInput").ap()
    xv_t = nc.dram_tensor("xv_t", [E, T], BF16, kind="ExternalInput").ap()
    wq_t = nc.dram_tensor("wq_t", [E, EH], BF16, kind="ExternalInput").ap()
    wk_t = nc.dram_tensor("wk_t", [E, EH], BF16, kind="ExternalInput").ap()
    wv_t = nc.dram_tensor("wv_t", [E, EH], BF16, kind="ExternalInput").ap()
    wo_t = nc.dram_tensor("wo_t", [EH, E], BF16, kind="ExternalInput").ap()
    bq_d = nc.dram_tensor("bq_d", [P, KHD], F32, kind="ExternalInput").ap()
    bk_d = nc.dram_tensor("bk_d", [P, KHD], F32, kind="ExternalInput").ap()
    bv_d = nc.dram_tensor("bv_d", [P, EH], F32, kind="ExternalInput").ap()
    mask_d = nc.dram_tensor("mask_d", [P, 4, TC], BF16, kind="ExternalInput").ap()
    out_p = nc.dram_tensor("out_part", [T, E], F32, kind="ExternalOutput").ap()

    from contextlib import ExitStack

    with tile.TileContext(nc) as tc, ExitStack() as ctx:
        persist = ctx.enter_context(tc.tile_pool(name="persist", bufs=1))
        evac = ctx.enter_context(tc.tile_pool(name="evac", bufs=4))

        nc.gpsimd.load_library(library_config.attn)  # for partition_broadcast

        # ---- persistent tiles
        wq_sb = persist.tile([P, KO, EH], BF16, tag="wq")
        wk_sb = persist.tile([P, KO, EH], BF16, tag="wk")
        wv_sb = persist.tile([P, KO, EH], BF16, tag="wv")
        bq_sb = persist.tile([P, KHD], F32, tag="bq")
        bk_sb = persist.tile([P, KHD], F32, tag="bk")
        bv_sb = persist.tile([P, EH], F32, tag="bv")
        mask_sb = persist.tile([P, 4, TC], BF16, tag="mask")
        wo_sb = persist.tile([P, KHD, E], BF16, tag="wo")
        qt_sb = persist.tile([P, KHD, T], BF16, tag="qt")
        kt_sb = persist.tile([P, KHD, T], BF16, tag="kt")
        v_sb = persist.tile([P, NSB, HC * VW], BF16, tag="v")
        attnT = persist.tile([P, KHD, T], BF16, tag="attnT")
        warm_sb = persist.tile([P, TC], BF16, tag="warm")

        # DMA queue round-robin. Weights go on sync+scalar (early, before the
        # scalar queue fills with exps); x chunks go on sync+gpsimd so their
        # triggers (~0.6us each) never delay ACT work.
        dma_rr = [0, 0]

        def dma(dst, src):
            eng = nc.sync if dma_rr[0] % 2 == 0 else nc.scalar
            dma_rr[0] += 1
            eng.dma_start(dst, src)

        def dmax(dst, src):
            eng = nc.sync if dma_rr[1] % 2 == 0 else nc.gpsimd
            dma_rr[1] += 1
            eng.dma_start(dst, src)

        wq_src = wq_t.rearrange("(ko p) m -> p ko m", p=P)
        wk_src = wk_t.rearrange("(ko p) m -> p ko m", p=P)
        wv_src = wv_t.rearrange("(ko p) m -> p ko m", p=P)
        xq_src = xq_t.rearrange("(ko p) t -> p ko t", p=P)
        xk_src = xk_t.rearrange("(ko p) t -> p ko t", p=P)
        xv_src = xv_t.rearrange("(ko p) t -> p ko t", p=P)
        wo_src = wo_t.rearrange("(ko p) m -> p ko m", p=P)

        # ---- single fused phase: attention score groups stream in
        # 64-row-tile mode (head pairs concurrent on PE row-tiles T0/T8);
        # ALL 128-mode matmul work (Q/K/V projections, PV accumulation,
        # out-projection) flows through a deferred FIFO popped between score
        # groups, keeping both PE and ACT saturated from ~10us onward.
        with (
            tc.tile_pool(name="xr", bufs=2) as xr,
            tc.tile_pool(name="exps", bufs=16) as exps,
            tc.tile_pool(name="psS", bufs=2, space="PSUM") as psS,
            tc.tile_pool(name="psPV", bufs=2, space="PSUM") as psPV,
        ):
            # x ring tiles: each input streams through 2 ring slots of
            # [P, KO, TC]; tj0/tj1 transfers are emitted upfront, tj2/tj3 via
            # FIFO units (after their ring slot's readers are emitted).
            xq_tiles = [
                xr.tile([P, KO, TC], BF16, tag="xq", name=f"xq{tj}")
                for tj in range(NT)
            ]
            xk_tiles = [
                xr.tile([P, KO, TC], BF16, tag="xk", name=f"xk{tj}")
                for tj in range(NT)
            ]
            xv_tiles = [
                xr.tile([P, KO, TC], BF16, tag="xv", name=f"xv{tj}")
                for tj in range(NT)
            ]

            def make_xdma(tiles, src, tj):
                def _emit():
                    for ko in range(KO):
                        dmax(
                            tiles[tj][:, ko, :],
                            src[:, ko, tj * TC : (tj + 1) * TC],
                        )

                return _emit
            pe128 = []  # deferred FIFO of (unit_id, emit_fn)
            emitted = set()
            uid_counter = [0]

            def push(fn):
                uid = uid_counter[0]
                uid_counter[0] += 1
                pe128.append((uid, fn))
                return uid

            def pop_unit():
                uid, fn = pe128.pop(0)
                fn()
                emitted.add(uid)

            def pop_to(depth):
                # Keep the deferred queue bounded so PSUM/SBUF ring reuse
                # never outruns the emission of a tile's consumers.
                while len(pe128) > depth:
                    pop_unit()

            def drain_to(uid):
                while uid not in emitted:
                    pop_unit()

            def make_proj_unit(pair, tj, x_tiles, w_sb, b_sb, dst):
                def _emit():
                    ps = psPV.tile([P, TC], F32, tag="po")
                    for ko in range(KO):
                        nc.tensor.matmul(
                            ps[:],
                            lhsT=w_sb[:, ko, pair * P : (pair + 1) * P],
                            rhs=x_tiles[tj][:, ko, :],
                            start=(ko == 0),
                            stop=(ko == KO - 1),
                        )
                    nc.vector.tensor_scalar_add(
                        dst[:, pair, tj * TC : (tj + 1) * TC],
                        ps[:],
                        b_sb[:, pair : pair + 1],
                    )

                return _emit

            def make_v_unit(sb):
                def _emit():
                    ps = psPV.tile([P, EH], F32, tag="po")
                    for ko in range(KO):
                        nc.tensor.matmul(
                            ps[:],
                            lhsT=xv_tiles[sb // 4][
                                :, ko, (sb % 4) * P : (sb % 4 + 1) * P
                            ],
                            rhs=wv_sb[:, ko, :],
                            start=(ko == 0),
                            stop=(ko == KO - 1),
                        )
                    v_dst = v_sb[:, sb, :].rearrange("p (h x) -> p h x", h=HC)[
                        :, :, 0:D
                    ]
                    nc.vector.tensor_add(
                        v_dst,
                        ps[:].rearrange("p (h x) -> p h x", h=HC),
                        bv_sb[:].rearrange("p (h x) -> p h x", h=HC),
                    )

                return _emit

            def make_pv_pair(pvA, pvB, si, hA, hB, ets, tj):
                last_si = 4 * tj + 3

                def _emit():
                    et = ets.pop(si)
                    for pv, h, u in ((pvA, hA, 0), (pvB, hB, 1)):
                        nc.tensor.matmul(
                            pv[0 : D + 1, :],
                            lhsT=v_sb[:, si, h * VW : (h + 1) * VW],
                            rhs=et[:, u, :],
                            start=(si == 0),
                            stop=(si == last_si),
                        )

                return _emit

            def make_norm(pv, pb, pair, tj):
                def _emit():
                    # reciprocal_approx_fast reads partition 0 regardless of
                    # the input AP's base partition, so stage the den row
                    # (PSUM partition 64) to a partition-0 tile first.
                    den0 = evac.tile([1, TC], F32, tag="den0", bufs=2)
                    nc.vector.tensor_copy(den0[:], pv[D : D + 1, :])
                    rec = evac.tile([1, TC], F32, tag="rec", bufs=2)
                    nc.vector.reciprocal_approx_fast(rec[:], den0[:])
                    rbs = evac.tile([D, TC], F32, tag="rbs", bufs=2)
                    nc.gpsimd.partition_broadcast(rbs[:], rec[:])
                    nc.vector.tensor_mul(
                        attnT[pb : pb + D, pair, tj * TC : (tj + 1) * TC],
                        pv[0:D, :],
                        rbs[:],
                    )

                return _emit

            def make_outproj(tb, fj):
                def _emit():
                    po = psPV.tile([P, TC], F32, tag="po")
                    for ko in range(KHD):
                        nc.tensor.matmul(
                            po[:],
                            lhsT=attnT[:, ko, tb * P : (tb + 1) * P],
                            rhs=wo_sb[:, ko, fj * TC : (fj + 1) * TC],
                            start=(ko == 0),
                            stop=(ko == KHD - 1),
                        )
                    ot = evac.tile([P, TC], F32, tag="ot", bufs=3)
                    nc.vector.tensor_copy(ot[:], po[:])
                    nc.gpsimd.dma_start(
                        out_p[tb * P : (tb + 1) * P, fj * TC : (fj + 1) * TC], ot[:]
                    )

                return _emit

            # startup: warm the HAM clock gate with junk matmuls while the
            # first weight DMAs land; interleave the first Q/K units with
            # the weight-DMA emission so their x transfers queue early.
            nc.vector.memset(warm_sb[:], 0.0)
            wps = psPV.tile([P, TC], F32, tag="po")
            for i in range(8):
                nc.tensor.matmul(
                    wps[:],
                    lhsT=warm_sb[:, 0:P],
                    rhs=warm_sb[:],
                    start=True,
                    stop=True,
                )
            for h in range(HC):
                nc.vector.memset(v_sb[:, :, h * VW + D : h * VW + VW], 1.0)

            batches = [(tj, pair) for tj in range(NT) for pair in range(NPAIR)]
            qk_marker = {}
            for ko in range(KO):
                dma(wq_sb[:, ko, :], wq_src[:, ko, :])
            dma(bq_sb[:], bq_d)
            make_xdma(xq_tiles, xq_src, 0)()
            for ko in range(KO):
                dma(wk_sb[:, ko, :], wk_src[:, ko, :])
            dma(bk_sb[:], bk_d)
            make_xdma(xk_tiles, xk_src, 0)()
            uid_q0 = push(make_proj_unit(0, 0, xq_tiles, wq_sb, bq_sb, qt_sb))
            drain_to(uid_q0)
            uid_k0 = push(make_proj_unit(0, 0, xk_tiles, wk_sb, bk_sb, kt_sb))
            drain_to(uid_k0)
            qk_marker[0] = (uid_q0, uid_k0)
            for ko in range(KO):
                dma(wv_sb[:, ko, :], wv_src[:, ko, :])
            dma(bv_sb[:], bv_d)
            dma(mask_sb[:], mask_d)
            make_xdma(xv_tiles, xv_src, 0)()
            make_xdma(xq_tiles, xq_src, 1)()
            make_xdma(xk_tiles, xk_src, 1)()
            make_xdma(xv_tiles, xv_src, 1)()
            for sb in range(4):
                push(make_v_unit(sb))
            for ko in range(KHD):
                dma(wo_sb[:, ko, :], wo_src[:, ko, :])

            for b, (tj, pair) in enumerate(batches):
                ng2 = 4 * tj + 4  # s-blocks in the causal span of this tj
                hA, hB = 2 * pair, 2 * pair + 1
                if True:
                    # push the NEXT batch's Q/K prereqs (one batch of lead so
                    # their x DMAs land before the matmuls need them), plus
                    # one V unit of the NEXT tj band
                    if b + 1 < len(batches):
                        tjn, pn = batches[b + 1]
                        qk_marker[b + 1] = (
                            push(
                                make_proj_unit(pn, tjn, xq_tiles, wq_sb, bq_sb, qt_sb)
                            ),
                            push(
                                make_proj_unit(pn, tjn, xk_tiles, wk_sb, bk_sb, kt_sb)
                            ),
                        )
                    sbv = 4 * tj + 4 + pair
                    if sbv < NSB:
                        push(make_v_unit(sbv))
                    if pair == NPAIR - 1 and tj + 2 < NT:
                        # ring slot for x[tj+2] frees once the last readers of
                        # x[tj] are emitted (Q/K(3, tj) above, V of band tj)
                        push(make_xdma(xq_tiles, xq_src, tj + 2))
                        push(make_xdma(xk_tiles, xk_src, tj + 2))
                        push(make_xdma(xv_tiles, xv_src, tj + 2))
                    # ensure this batch's own Q/K projections are emitted
                    for uid in qk_marker[b]:
                        drain_to(uid)
                    pvA = psPV.tile([P, TC], F32, tag="pv")
                    pvB = psPV.tile([P, TC], F32, tag="pv")
                    ets: dict = {}
                    for si in range(ng2):
                        # one group = one s-block for BOTH heads of the pair:
                        # a single PSUM tile + a single exp gate both next
                        # matmuls, so the scheduler keeps the A/B row-tile
                        # matmuls adjacent and they stream concurrently.
                        sc = psS.tile([P, 2, TC], F32, tag="sc")
                        for pb, u in ((0, 0), (D, 1)):
                            nc.tensor.matmul(
                                sc[:, u, :],
                                lhsT=kt_sb[pb : pb + D, pair, si * P : (si + 1) * P],
                                rhs=qt_sb[pb : pb + D, pair, tj * TC : (tj + 1) * TC],
                                start=True,
                                stop=True,
                            )
                        et = exps.tile([P, 2, TC], BF16, tag="et")
                        k = si - 4 * tj
                        if k >= 0:  # diagonal s-block: partial-span exp + mask
                            if k > 0:
                                nc.gpsimd.memset(et[:, :, 0 : P * k], 0.0)
                            nc.scalar.activation(
                                et[:, :, P * k : TC],
                                sc[:, :, P * k : TC],
                                AF.Exp,
                                scale=1.0 / math.sqrt(D),
                            )
                            for u in range(2):
                                nc.vector.tensor_mul(
                                    et[:, u, P * k : P * (k + 1)],
                                    et[:, u, P * k : P * (k + 1)],
                                    mask_sb[:, k, P * k : P * (k + 1)],
                                )
                        else:
                            nc.scalar.activation(
                                et[:], sc[:], AF.Exp, scale=1.0 / math.sqrt(D)
                            )
                        ets[si] = et
                        push(make_pv_pair(pvA, pvB, si, hA, hB, ets, tj))
                        # pop deferred 128-mode work in small batches (batching
                        # limits PE tiling-mode switches between the 64-row
                        # score stream and the 128-row PV/out-proj stream)
                        if si % 3 == 2:
                            pop_to(3)
                    push(make_norm(pvA, 0, pair, tj))
                    push(make_norm(pvB, D, pair, tj))
                if pair == NPAIR - 1:
                    # queue this tj's out-projection tiles; they run inside tj+1
                    for tb in range(4 * tj, 4 * tj + 4):
                        for fj in range(E // TC):
                            push(make_outproj(tb, fj))
            pop_to(0)

    nc.compile()
    return nc


def _get_nc():
    if "nc" not in _CACHE:
        _CACHE["nc"] = _build_nc()
    return _CACHE["nc"]


def _prep_in_maps(query, key, value, attn_mask, Wq, bq, Wk, bk, Wv, bv, Wo, bo):
    """Host-side prep: slices, transposes, bf16 casts. Returns in_maps[8]."""
    f32 = np.float32
    xt = {}  # (kind, b) -> [E, T] bf16
    for b in range(B):
        xt[("q", b)] = np.ascontiguousarray(query[:, b, :].T).astype(NPBF16)
        xt[("k", b)] = np.ascontiguousarray(key[:, b, :].T).astype(NPBF16)
        xt[("v", b)] = np.ascontiguousarray(value[:, b, :].T).astype(NPBF16)
    wt = {}
    for hg in range(2):
        sl = slice(EH * hg, EH * hg + EH)
        wt[("q", hg)] = np.ascontiguousarray(Wq[sl, :].T).astype(NPBF16)
        wt[("k", hg)] = np.ascontiguousarray(Wk[sl, :].T).astype(NPBF16)
        wt[("v", hg)] = np.ascontiguousarray(Wv[sl, :].T).astype(NPBF16)
        wt[("o", hg)] = np.ascontiguousarray(Wo[:, sl].T).astype(NPBF16)
        wt[("bq", hg)] = np.ascontiguousarray(
            bq[sl].astype(f32).reshape(KHD, P).T
        )
        wt[("bk", hg)] = np.ascontiguousarray(
            bk[sl].astype(f32).reshape(KHD, P).T
        )
        wt[("bv", hg)] = np.ascontiguousarray(
            np.tile(bv[sl].astype(f32)[None, :], (P, 1))
        )
    # mask patterns: for a scores tile with s0 = t0 + 128*o, pattern
    # [p, o, f] = 0 if attn_mask[t0+f, s0+p] (masked) else 1.
    t0 = 512
    patts = []
    for o in range(4):
        s0 = t0 + P * o
        patts.append(
            (~np.asarray(attn_mask[t0 : t0 + TC, s0 : s0 + P])).T.astype(NPBF16)
        )
    mask_tiles = np.ascontiguousarray(np.stack(patts, axis=1))  # [P, 4, TC]

    in_maps = []
    for c in range(NCORES):
        b, hg = c // 2, c % 2
        in_maps.append(
            {
                "xq_t": xt[("q", b)],
                "xk_t": xt[("k", b)],
                "xv_t": xt[("v", b)],
                "wq_t": wt[("q", hg)],
                "wk_t": wt[("k", hg)],
                "wv_t": wt[("v", hg)],
                "wo_t": wt[("o", hg)],
                "bq_d": wt[("bq", hg)],
                "bk_d": wt[("bk", hg)],
                "bv_d": wt[("bv", hg)],
                "mask_d": mask_tiles,
            }
        )
    return in_maps


def _run_on_hw(in_maps, trace=False, **kwargs):
    nc = _get_nc()
    return bass_utils.run_bass_kernel_spmd(
        nc, in_maps, core_ids=list(range(NCORES)), trace=trace, **kwargs
    )


def _gather(results, bo):
    outs = []
    for b in range(B):
        part = results[2 * b]["out_part"] + results[2 * b + 1]["out_part"]
        outs.append(part)
    out = np.stack(outs, axis=1)  # [T, B, E]
    out += np.asarray(bo, dtype=np.float32)[None, None, :]
    return out.astype(np.float32)


def _numpy_fallback(query, key, value, attn_mask, Wq, bq, Wk, bk, Wv, bv, Wo, bo):
    """Exact f32 numpy replication of the reference (for non-causal masks)."""
    f32 = np.float32
    query, key, value = (np.asarray(a, f32) for a in (query, key, value))
    q = (np.einsum("tbe,fe->btf", query, Wq, dtype=f32) + bq).reshape(B, T, H, D)
    k = (np.einsum("sbe,fe->bsf", key, Wk, dtype=f32) + bk).reshape(B, S, H, D)
    v = (np.einsum("sbe,fe->bsf", value, Wv, dtype=f32) + bv).reshape(B, S, H, D)
    q, k, v = (a.transpose(0, 2, 1, 3) for a in (q, k, v))
    out = np.empty((B, H, T, D), f32)
    mask = np.asarray(attn_mask)
    for b in range(B):
        for h in range(H):
            sc = (q[b, h] @ k[b, h].T) / np.float32(math.sqrt(D))
            sc = np.where(mask, -np.inf, sc)
            m = np.max(sc, axis=-1, keepdims=True)
            m = np.where(np.isfinite(m), m, 0.0)
            e = np.exp(sc - m)
            p = e / np.sum(e, axis=-1, keepdims=True)
            p = np.where(np.isinf(sc), 0.0, p)
            out[b, h] = p @ v[b, h]
    out = out.transpose(0, 2, 1, 3).reshape(B, T, E)
    out = out @ np.asarray(Wo, f32).T + bo
    return np.ascontiguousarray(out.transpose(1, 0, 2)).astype(f32)


def kernel(query, key, value, attn_mask, Wq, bq, Wk, bk, Wv, bv, Wo, bo):
    mask = np.asarray(attn_mask)
    causal = mask.shape == (T, S) and np.array_equal(
        mask, np.triu(np.ones((T, S), dtype=bool), k=1)
    )
    if not causal:
        return _numpy_fallback(
            query, key, value, attn_mask, Wq, bq, Wk, bk, Wv, bv, Wo, bo
        )
    in_maps = _prep_in_maps(
        query, key, value, attn_mask, Wq, bq, Wk, bk, Wv, bv, Wo, bo
    )
    res = _run_on_hw(in_maps)
    return _gather(res.results, bo)
